# revision 2
# baseline (speedup 1.0000x reference)
"""Trainium2 Bass kernel for nn_BackboneCurvatureMixer.

Strategy: data-parallel over batch B=8 (one sequence per NeuronCore), SPMD.
On-chip layout is feature-major ("transposed activations"): activations live
as (feature partitions, token free-dim) so chained matmuls need no on-chip
transposes; the host pre-transposes h and post-transposes all outputs (free).

Per core pipeline (L=4096 tokens, chunked by T=512):
  zT = w_red^T-contract(hT)  (PE, bf16)            -> (32, L)
  ZA/ZB = pair-gather of zT rows via 0/1 matrices  (PE)  -> (496, T+4)
  ||p||^2 via Lagrange identity |u|^2|v|^2-(u.v)^2 (PE ones-matmul + DVE)
  plucker products + normalize                     (DVE)
  kappa = second difference along tokens           (DVE, free-axis shifts)
  two MLPs (1488->512->1024, 496->512->1024)       (PE bf16, exact-GELU on ACT)
"""

import numpy as np
import ml_dtypes

import concourse.bass as bass
import concourse.mybir as mybir
import concourse.tile as tile
import concourse.bacc as bacc
from concourse.bass_utils import run_bass_kernel_spmd

BF = ml_dtypes.bfloat16
DT = mybir.dt
AF = mybir.ActivationFunctionType
ALU = mybir.AluOpType

B, L, D, R, HID = 8, 4096, 1024, 32, 512
PLU = R * (R - 1) // 2          # 496
NPK, PK = 4, 124                # 496 = 4 x 124 partition tiles
T = 512                         # token chunk
NT = L // T
LP = L + 8                      # padded token length (zero tail for shifts)
DC = D // 128                   # 8 d-chunks
OFFS = (1, 2, 4)

# knobs for test harness
TRACE = False
TRACE_DIR = None
LAST_RESULT = None

_CACHE = {}


def _build(with_mask: bool):
    f32, bf16 = DT.float32, DT.bfloat16
    nc = bacc.Bacc(None, target_bir_lowering=False, debug=False)

    HT = nc.dram_tensor("hT", [D, L], bf16, kind="ExternalInput")
    WREDT = nc.dram_tensor("wredT", [D, R], bf16, kind="ExternalInput")
    BRED = nc.dram_tensor("bred", [R, 1], f32, kind="ExternalInput")
    GA = nc.dram_tensor("ga", [R, PLU], bf16, kind="ExternalInput")
    GB = nc.dram_tensor("gb", [R, PLU], bf16, kind="ExternalInput")
    W1T = nc.dram_tensor("w1T", [3 * PLU, HID], bf16, kind="ExternalInput")
    B1 = nc.dram_tensor("b1", [HID, 1], f32, kind="ExternalInput")
    W2T = nc.dram_tensor("w2T", [HID, D], bf16, kind="ExternalInput")
    B2 = nc.dram_tensor("b2", [D, 1], f32, kind="ExternalInput")
    C1T = nc.dram_tensor("c1T", [PLU, HID], bf16, kind="ExternalInput")
    CB1 = nc.dram_tensor("cb1", [HID, 1], f32, kind="ExternalInput")
    C2T = nc.dram_tensor("c2T", [HID, D], bf16, kind="ExternalInput")
    CB2 = nc.dram_tensor("cb2", [D, 1], f32, kind="ExternalInput")
    MSK = (
        nc.dram_tensor("mask128", [128, LP], bf16, kind="ExternalInput")
        if with_mask
        else None
    )

    ZOUT = nc.dram_tensor("zT_out", [R, L], f32, kind="ExternalOutput")
    P1OUT = nc.dram_tensor("p1T_out", [PLU, L], bf16, kind="ExternalOutput")
    KOUT = nc.dram_tensor("kT_out", [PLU, L], bf16, kind="ExternalOutput")
    GBOUT = nc.dram_tensor("gbT_out", [D, L], bf16, kind="ExternalOutput")
    GCOUT = nc.dram_tensor("gcT_out", [D, L], bf16, kind="ExternalOutput")

    with tile.TileContext(nc) as tc:
        with (
            tc.tile_pool(name="persist", bufs=1) as pers,
            tc.tile_pool(name="wts", bufs=1) as wts,
            tc.tile_pool(name="hin", bufs=2) as hin,
            tc.tile_pool(name="gath", bufs=2) as gath,
            tc.tile_pool(name="nrm", bufs=2) as nrm,
            tc.tile_pool(name="scr", bufs=6) as scr,
            tc.tile_pool(name="fpl", bufs=2) as fpl,
            tc.tile_pool(name="plk", bufs=4) as plk,
            tc.tile_pool(name="pmt", bufs=2) as pmt,
            tc.tile_pool(name="g1p", bufs=6) as g1p,
            tc.tile_pool(name="stg", bufs=2) as stg,
            tc.tile_pool(name="ps", bufs=6, space="PSUM") as psp,
            tc.tile_pool(name="pst", bufs=2, space="PSUM") as pstp,
        ):
            # ---- weights into SBUF (once) ----
            wred_sb = wts.tile([128, DC, R], bf16, tag="wred")
            nc.sync.dma_start(
                out=wred_sb[:], in_=WREDT[:].rearrange("(c p) r -> p c r", p=128)
            )
            bred_sb = wts.tile([R, 1], f32, tag="bred")
            nc.sync.dma_start(out=bred_sb[:], in_=BRED[:])
            ga_sb = wts.tile([R, PLU], bf16, tag="ga")
            nc.sync.dma_start(out=ga_sb[:], in_=GA[:])
            gb_sb = wts.tile([R, PLU], bf16, tag="gb")
            nc.sync.dma_start(out=gb_sb[:], in_=GB[:])
            ones_sb = wts.tile([R, 128], bf16, tag="ones")
            nc.vector.memset(ones_sb[:], 1.0)
            w1_sb = wts.tile([PK, 12, HID], bf16, tag="w1")
            nc.sync.dma_start(
                out=w1_sb[:], in_=W1T[:].rearrange("(k p) h -> p k h", p=PK)
            )
            b1_sb = wts.tile([128, 4], f32, tag="b1")
            nc.sync.dma_start(
                out=b1_sb[:], in_=B1[:].rearrange("(m p) o -> p (m o)", p=128)
            )
            w2_sb = wts.tile([128, 4, D], bf16, tag="w2")
            nc.sync.dma_start(
                out=w2_sb[:], in_=W2T[:].rearrange("(k p) d -> p k d", p=128)
            )
            b2_sb = wts.tile([128, DC], f32, tag="b2")
            nc.sync.dma_start(
                out=b2_sb[:], in_=B2[:].rearrange("(m p) o -> p (m o)", p=128)
            )
            c1_sb = wts.tile([PK, NPK, HID], bf16, tag="c1")
            nc.sync.dma_start(
                out=c1_sb[:], in_=C1T[:].rearrange("(k p) h -> p k h", p=PK)
            )
            cb1_sb = wts.tile([128, 4], f32, tag="cb1")
            nc.sync.dma_start(
                out=cb1_sb[:], in_=CB1[:].rearrange("(m p) o -> p (m o)", p=128)
            )
            c2_sb = wts.tile([128, 4, D], bf16, tag="c2")
            nc.sync.dma_start(
                out=c2_sb[:], in_=C2T[:].rearrange("(k p) d -> p k d", p=128)
            )
            cb2_sb = wts.tile([128, DC], f32, tag="cb2")
            nc.sync.dma_start(
                out=cb2_sb[:], in_=CB2[:].rearrange("(m p) o -> p (m o)", p=128)
            )
            if with_mask:
                mask_sb = pers.tile([128, LP], bf16, tag="msk")
                nc.sync.dma_start(out=mask_sb[:], in_=MSK[:])

            # ---- persistent activations ----
            zTb = pers.tile([R, LP], bf16, tag="zTb")
            nc.vector.memset(zTb[:, L:LP], 0.0)
            if with_mask:
                zTbm = pers.tile([R, LP], bf16, tag="zTbm")
                nc.vector.memset(zTbm[:, L:LP], 0.0)
            else:
                zTbm = zTb
            p1t = []
            for mk in range(NPK):
                pt = pers.tile([PK, LP], bf16, tag=f"p1_{mk}")
                nc.vector.memset(pt[:, 0:1], 0.0)
                nc.vector.memset(pt[:, L + 1 : LP], 0.0)
                p1t.append(pt)

            # ---- phase Z: z for all chunks ----
            for t in range(NT):
                t0 = t * T
                ht = hin.tile([128, DC, T], bf16, tag="ht")
                nc.sync.dma_start(
                    out=ht[:],
                    in_=HT[:].rearrange("(c p) l -> p c l", p=128)[:, :, t0 : t0 + T],
                )
                zp = psp.tile([R, T], f32, tag="mm")
                for c in range(DC):
                    nc.tensor.matmul(
                        zp[:],
                        wred_sb[:, c, :],
                        ht[:, c, :],
                        start=(c == 0),
                        stop=(c == DC - 1),
                    )
                zst = nrm.tile([R, T], f32, tag="zst")
                nc.scalar.activation(
                    out=zst[:], in_=zp[:], func=AF.Identity, bias=bred_sb[:], scale=1.0
                )
                nc.sync.dma_start(out=ZOUT[:][:, t0 : t0 + T], in_=zst[:])
                nc.scalar.activation(
                    out=zTb[:, t0 : t0 + T],
                    in_=zp[:],
                    func=AF.Identity,
                    bias=bred_sb[:],
                    scale=1.0,
                )
                if with_mask:
                    nc.vector.tensor_mul(
                        zTbm[:, t0 : t0 + T],
                        zTb[:, t0 : t0 + T],
                        mask_sb[0:R, t0 : t0 + T],
                    )

            def emit_main(t):
                t0 = t * T
                # gathers: ZA[k,l]=z[ia_k,l], ZB[k,l]=z[ib_k,l] (masked z)
                zarrs = {"a": [], "b": []}
                for nm, gsb in (("a", ga_sb), ("b", gb_sb)):
                    for mk in range(NPK):
                        pm_ = psp.tile([PK, T], f32, tag="mm")
                        nc.tensor.matmul(
                            pm_[:],
                            gsb[:, mk * PK : (mk + 1) * PK],
                            zTbm[:, t0 : t0 + T],
                            start=True,
                            stop=True,
                        )
                        pt_ = pstp.tile([PK, 4], f32, tag="mmt")
                        nc.tensor.matmul(
                            pt_[:],
                            gsb[:, mk * PK : (mk + 1) * PK],
                            zTbm[:, t0 + T : t0 + T + 4],
                            start=True,
                            stop=True,
                        )
                        g = gath.tile([PK, T + 4], bf16, tag=f"g{nm}{mk}")
                        nc.scalar.activation(out=g[:, 0:T], in_=pm_[:], func=AF.Copy)
                        nc.scalar.activation(
                            out=g[:, T : T + 4], in_=pt_[:], func=AF.Copy
                        )
                        zarrs[nm].append(g)
                ZAt, ZBt = zarrs["a"], zarrs["b"]
                # norms via Lagrange identity (raw z)
                z2 = nrm.tile([R, T + 4], bf16, tag="z2")
                nc.vector.tensor_mul(
                    z2[:], zTb[:, t0 : t0 + T + 4], zTb[:, t0 : t0 + T + 4]
                )
                s2ps = psp.tile([128, T], f32, tag="mm")
                nc.tensor.matmul(s2ps[:], ones_sb[:], z2[:, 0:T], start=True, stop=True)
                s2sb = nrm.tile([128, T], f32, tag="s2sb")
                nc.scalar.activation(out=s2sb[:], in_=s2ps[:], func=AF.Copy)
                Fs = {}
                for d in OFFS:
                    dm = nrm.tile([R, T], bf16, tag="dm")
                    nc.vector.tensor_mul(
                        dm[:], zTb[:, t0 : t0 + T], zTb[:, t0 + d : t0 + d + T]
                    )
                    ssps = psp.tile([128, T], f32, tag="mm")
                    nc.tensor.matmul(
                        ssps[:], ones_sb[:], z2[:, d : d + T], start=True, stop=True
                    )
                    dps = psp.tile([128, T], f32, tag="mm")
                    nc.tensor.matmul(dps[:], ones_sb[:], dm[:], start=True, stop=True)
                    v = scr.tile([128, T], f32, tag="scr")
                    nc.vector.tensor_mul(v[:], s2sb[:], ssps[:])
                    e = scr.tile([128, T], f32, tag="scr")
                    nc.scalar.activation(out=e[:], in_=dps[:], func=AF.Square)
                    nc.vector.tensor_sub(v[:], v[:], e[:])
                    nc.vector.tensor_scalar_max(v[:], v[:], 1e-16)
                    r = scr.tile([128, T], f32, tag="scr")
                    nc.vector.reciprocal_approx_fast(r[:], v[:])
                    Ft = fpl.tile([128, T], bf16, tag=f"F{d}")
                    nc.scalar.activation(out=Ft[:], in_=r[:], func=AF.Sqrt)
                    Fs[d] = Ft
                # plucker products, normalize, write p tiles
                pms = {2: [], 4: []}
                for d in OFFS:
                    for mk in range(NPK):
                        t1 = plk.tile([PK, T], bf16, tag="t1")
                        nc.vector.tensor_mul(
                            t1[:], ZAt[mk][:, 0:T], ZBt[mk][:, d : d + T]
                        )
                        t2 = plk.tile([PK, T], bf16, tag="t2")
                        nc.vector.tensor_mul(
                            t2[:], ZBt[mk][:, 0:T], ZAt[mk][:, d : d + T]
                        )
                        nc.vector.tensor_sub(t1[:], t1[:], t2[:])
                        if d == 1:
                            dest = p1t[mk][:, t0 + 1 : t0 + T + 1]
                        else:
                            pm = pmt.tile([PK, T], bf16, tag=f"pm{d}_{mk}")
                            pms[d].append(pm)
                            dest = pm[:]
                        nc.vector.tensor_mul(dest, t1[:], Fs[d][0:PK, :])
                # MLP1+MLP2 (backbone)
                rhs = [p1t[mk][:, t0 + 1 : t0 + T + 1] for mk in range(NPK)]
                rhs += [pm[:] for pm in pms[2]]
                rhs += [pm[:] for pm in pms[4]]
                g1s = []
                for m in range(4):
                    y = psp.tile([128, T], f32, tag="mm")
                    for kc in range(12):
                        nc.tensor.matmul(
                            y[:],
                            w1_sb[:, kc, m * 128 : (m + 1) * 128],
                            rhs[kc],
                            start=(kc == 0),
                            stop=(kc == 11),
                        )
                    g1 = g1p.tile([128, T], bf16, tag="g1")
                    nc.scalar.activation(
                        out=g1[:],
                        in_=y[:],
                        func=AF.Gelu,
                        bias=b1_sb[:, m : m + 1],
                        scale=1.0,
                    )
                    g1s.append(g1)
                gstage = stg.tile([128, DC, T], bf16, tag="gst")
                for m in range(DC):
                    y2 = psp.tile([128, T], f32, tag="mm")
                    for k in range(4):
                        nc.tensor.matmul(
                            y2[:],
                            w2_sb[:, k, m * 128 : (m + 1) * 128],
                            g1s[k][:],
                            start=(k == 0),
                            stop=(k == 3),
                        )
                    nc.scalar.activation(
                        out=gstage[:, m, :],
                        in_=y2[:],
                        func=AF.Identity,
                        bias=b2_sb[:, m : m + 1],
                        scale=1.0,
                    )
                nc.sync.dma_start(
                    out=GBOUT[:].rearrange("(c p) l -> p c l", p=128)[
                        :, :, t0 : t0 + T
                    ],
                    in_=gstage[:],
                )

            def emit_curv(tp):
                tp0 = tp * T
                kst = stg.tile([PK, NPK, T], bf16, tag="kst")
                for mk in range(NPK):
                    ka = kst[:, mk, :]
                    nc.vector.tensor_add(
                        ka,
                        p1t[mk][:, tp0 + 2 : tp0 + T + 2],
                        p1t[mk][:, tp0 : tp0 + T],
                    )
                    nc.vector.scalar_tensor_tensor(
                        out=ka,
                        in0=p1t[mk][:, tp0 + 1 : tp0 + T + 1],
                        scalar=-2.0,
                        in1=ka,
                        op0=ALU.mult,
                        op1=ALU.add,
                    )
                    if with_mask:
                        nc.vector.tensor_mul(ka, ka, mask_sb[0:PK, tp0 : tp0 + T])
                nc.sync.dma_start(
                    out=KOUT[:].rearrange("(c p) l -> p c l", p=PK)[
                        :, :, tp0 : tp0 + T
                    ],
                    in_=kst[:],
                )
                g1s = []
                for m in range(4):
                    y = psp.tile([128, T], f32, tag="mm")
                    for kc in range(NPK):
                        nc.tensor.matmul(
                            y[:],
                            c1_sb[:, kc, m * 128 : (m + 1) * 128],
                            kst[:, kc, :],
                            start=(kc == 0),
                            stop=(kc == NPK - 1),
                        )
                    g1 = g1p.tile([128, T], bf16, tag="g1")
                    nc.scalar.activation(
                        out=g1[:],
                        in_=y[:],
                        func=AF.Gelu,
                        bias=cb1_sb[:, m : m + 1],
                        scale=1.0,
                    )
                    g1s.append(g1)
                cstage = stg.tile([128, DC, T], bf16, tag="cst")
                for m in range(DC):
                    y2 = psp.tile([128, T], f32, tag="mm")
                    for k in range(4):
                        nc.tensor.matmul(
                            y2[:],
                            c2_sb[:, k, m * 128 : (m + 1) * 128],
                            g1s[k][:],
                            start=(k == 0),
                            stop=(k == 3),
                        )
                    nc.scalar.activation(
                        out=cstage[:, m, :],
                        in_=y2[:],
                        func=AF.Identity,
                        bias=cb2_sb[:, m : m + 1],
                        scale=1.0,
                    )
                nc.sync.dma_start(
                    out=GCOUT[:].rearrange("(c p) l -> p c l", p=128)[
                        :, :, tp0 : tp0 + T
                    ],
                    in_=cstage[:],
                )

            for t in range(NT):
                emit_main(t)
                if t >= 1:
                    emit_curv(t - 1)
            emit_curv(NT - 1)

            for mk in range(NPK):
                nc.sync.dma_start(
                    out=P1OUT[:][mk * PK : (mk + 1) * PK, :],
                    in_=p1t[mk][:, 1 : L + 1],
                )

    nc.compile()
    return nc


def _prep_maps(h, seq_mask, w_red, b_red, bb_w1, bb_b1, bb_w2, bb_b2, cv_w1, cv_b1,
               cv_w2, cv_b2, with_mask):
    f = lambda x: np.ascontiguousarray(np.asarray(x, np.float32))
    bb = lambda x: np.ascontiguousarray(np.asarray(x, np.float32)).astype(BF)
    ia, ib = np.triu_indices(R, k=1)
    ga = np.zeros((R, PLU), np.float32)
    gb = np.zeros((R, PLU), np.float32)
    ga[ia, np.arange(PLU)] = 1.0
    gb[ib, np.arange(PLU)] = 1.0
    shared = {
        "wredT": bb(f(w_red).T),
        "bred": f(b_red).reshape(R, 1),
        "ga": ga.astype(BF),
        "gb": gb.astype(BF),
        "w1T": bb(f(bb_w1).T),
        "b1": f(bb_b1).reshape(HID, 1),
        "w2T": bb(f(bb_w2).T),
        "b2": f(bb_b2).reshape(D, 1),
        "c1T": bb(f(cv_w1).T),
        "cb1": f(cv_b1).reshape(HID, 1),
        "c2T": bb(f(cv_w2).T),
        "cb2": f(cv_b2).reshape(D, 1),
    }
    h = np.asarray(h, np.float32)
    maps = []
    for i in range(B):
        m = dict(shared)
        m["hT"] = np.ascontiguousarray(h[i].T).astype(BF)
        if with_mask:
            mf = np.zeros((LP,), np.float32)
            mf[:L] = np.asarray(seq_mask[i], np.float32)
            m["mask128"] = np.broadcast_to(mf[None, :], (128, LP)).astype(BF)
        maps.append(m)
    return maps


def kernel(h, seq_mask, w_red, b_red, bb_w1, bb_b1, bb_w2, bb_b2, cv_w1, cv_b1,
           cv_w2, cv_b2):
    global LAST_RESULT
    mask = np.asarray(seq_mask).astype(bool)
    with_mask = not bool(mask.all())
    nc = _CACHE.get(with_mask)
    if nc is None:
        nc = _build(with_mask)
        _CACHE[with_mask] = nc
    maps = _prep_maps(h, seq_mask, w_red, b_red, bb_w1, bb_b1, bb_w2, bb_b2,
                      cv_w1, cv_b1, cv_w2, cv_b2, with_mask)
    kw = {}
    if TRACE:
        kw = dict(trace=True, tmpdir=TRACE_DIR)
    res = run_bass_kernel_spmd(nc, maps, list(range(B)), **kw)
    LAST_RESULT = res
    rs = res.results
    z = np.stack([np.asarray(rs[i]["zT_out"], np.float32).T for i in range(B)])
    g_bb = np.stack(
        [np.asarray(rs[i]["gbT_out"]).T.astype(np.float32) for i in range(B)]
    )
    g_curv = np.stack(
        [np.asarray(rs[i]["gcT_out"]).T.astype(np.float32) for i in range(B)]
    )
    p_bb1 = np.stack(
        [np.asarray(rs[i]["p1T_out"]).T.astype(np.float32) for i in range(B)]
    )
    kappa = np.stack(
        [np.asarray(rs[i]["kT_out"]).T.astype(np.float32) for i in range(B)]
    )
    return z, g_bb, g_curv, p_bb1, kappa


# revision 5
# speedup vs baseline: 1.0417x; 1.0417x over previous
"""Trainium2 Bass kernel for nn_BackboneCurvatureMixer.

Strategy: data-parallel over batch B=8 (one sequence per NeuronCore), SPMD.
On-chip layout is feature-major ("transposed activations"): activations live
as (feature partitions, token free-dim) so chained matmuls need no on-chip
transposes; the host pre-transposes h and post-transposes all outputs (free).

Per core pipeline (L=4096 tokens, chunked by T=512):
  zT = w_red^T-contract(hT)  (PE, bf16)            -> (32, L)
  ZA/ZB = pair-gather of zT rows via 0/1 matrices  (PE)  -> (496, T+4)
  ||p||^2 via Lagrange identity |u|^2|v|^2-(u.v)^2 (PE ones-matmul + DVE)
  plucker products + normalize                     (DVE)
  kappa = second difference along tokens           (DVE, free-axis shifts)
  two MLPs (1488->512->1024, 496->512->1024)       (PE bf16, exact-GELU on ACT)
"""

import numpy as np
import ml_dtypes

import concourse.bass as bass
import concourse.mybir as mybir
import concourse.tile as tile
import concourse.bacc as bacc
from concourse.bass_utils import run_bass_kernel_spmd

BF = ml_dtypes.bfloat16
DT = mybir.dt
AF = mybir.ActivationFunctionType
ALU = mybir.AluOpType

B, L, D, R, HID = 8, 4096, 1024, 32, 512
PLU = R * (R - 1) // 2          # 496
NPK, PK = 4, 124                # 496 = 4 x 124 partition tiles
T = 512                         # token chunk
NT = L // T
LP = L + 8                      # padded token length (zero tail for shifts)
DC = D // 128                   # 8 d-chunks
OFFS = (1, 2, 4)

# knobs for test harness
TRACE = False
TRACE_DIR = None
LAST_RESULT = None

_CACHE = {}


def _build(with_mask: bool):
    f32, bf16 = DT.float32, DT.bfloat16
    nc = bacc.Bacc(None, target_bir_lowering=False, debug=False)

    HT = nc.dram_tensor("hT", [D, L], bf16, kind="ExternalInput")
    WREDT = nc.dram_tensor("wredT", [D, R], bf16, kind="ExternalInput")
    BRED = nc.dram_tensor("bred", [R, 1], f32, kind="ExternalInput")
    GA = nc.dram_tensor("ga", [R, PLU], bf16, kind="ExternalInput")
    GB = nc.dram_tensor("gb", [R, PLU], bf16, kind="ExternalInput")
    W1T = nc.dram_tensor("w1T", [3 * PLU, HID], bf16, kind="ExternalInput")
    B1 = nc.dram_tensor("b1", [HID, 1], f32, kind="ExternalInput")
    W2T = nc.dram_tensor("w2T", [HID, D], bf16, kind="ExternalInput")
    B2 = nc.dram_tensor("b2", [D, 1], f32, kind="ExternalInput")
    C1T = nc.dram_tensor("c1T", [PLU, HID], bf16, kind="ExternalInput")
    CB1 = nc.dram_tensor("cb1", [HID, 1], f32, kind="ExternalInput")
    C2T = nc.dram_tensor("c2T", [HID, D], bf16, kind="ExternalInput")
    CB2 = nc.dram_tensor("cb2", [D, 1], f32, kind="ExternalInput")
    MSK = (
        nc.dram_tensor("mask128", [128, LP], bf16, kind="ExternalInput")
        if with_mask
        else None
    )

    ZOUT = nc.dram_tensor("zT_out", [R, L], f32, kind="ExternalOutput")
    P1OUT = nc.dram_tensor("p1T_out", [PLU, L], bf16, kind="ExternalOutput")
    KOUT = nc.dram_tensor("kT_out", [PLU, L], bf16, kind="ExternalOutput")
    GBOUT = nc.dram_tensor("gbT_out", [D, L], bf16, kind="ExternalOutput")
    GCOUT = nc.dram_tensor("gcT_out", [D, L], bf16, kind="ExternalOutput")

    with tile.TileContext(nc) as tc:
        with (
            tc.tile_pool(name="persist", bufs=1) as pers,
            tc.tile_pool(name="wts", bufs=1) as wts,
            tc.tile_pool(name="hin", bufs=2) as hin,
            tc.tile_pool(name="gath", bufs=2) as gath,
            tc.tile_pool(name="nrm", bufs=2) as nrm,
            tc.tile_pool(name="scr", bufs=6) as scr,
            tc.tile_pool(name="fpl", bufs=2) as fpl,
            tc.tile_pool(name="plk", bufs=4) as plk,
            tc.tile_pool(name="pmt", bufs=2) as pmt,
            tc.tile_pool(name="g1p", bufs=6) as g1p,
            tc.tile_pool(name="stg", bufs=2) as stg,
            tc.tile_pool(name="ps", bufs=7, space="PSUM") as psp,
            tc.tile_pool(name="pst", bufs=1, space="PSUM") as pstp,
        ):
            # ---- weights into SBUF (once) ----
            wred_sb = wts.tile([128, DC, R], bf16, tag="wred")
            nc.sync.dma_start(
                out=wred_sb[:], in_=WREDT[:].rearrange("(c p) r -> p c r", p=128)
            )
            bred_sb = wts.tile([R, 1], f32, tag="bred")
            nc.sync.dma_start(out=bred_sb[:], in_=BRED[:])
            if with_mask:
                mask_sb = pers.tile([128, LP], bf16, tag="msk")
                nc.sync.dma_start(out=mask_sb[:], in_=MSK[:])

            # ---- persistent activations ----
            zTb = pers.tile([R, LP], bf16, tag="zTb")
            nc.vector.memset(zTb[:, L:LP], 0.0)
            if with_mask:
                zTbm = pers.tile([R, LP], bf16, tag="zTbm")
                nc.vector.memset(zTbm[:, L:LP], 0.0)
            else:
                zTbm = zTb
            p1t = []
            for mk in range(NPK):
                pt = pers.tile([PK, LP], bf16, tag=f"p1_{mk}")
                nc.vector.memset(pt[:, 0:1], 0.0)
                nc.vector.memset(pt[:, L + 1 : LP], 0.0)
                p1t.append(pt)

            # ---- phase Z: z for all chunks ----
            for t in range(NT):
                t0 = t * T
                ht = hin.tile([128, DC, T], bf16, tag="ht")
                nc.sync.dma_start(
                    out=ht[:],
                    in_=HT[:].rearrange("(c p) l -> p c l", p=128)[:, :, t0 : t0 + T],
                )
                zp = psp.tile([R, T], f32, tag="mm")
                for c in range(DC):
                    nc.tensor.matmul(
                        zp[:],
                        wred_sb[:, c, :],
                        ht[:, c, :],
                        start=(c == 0),
                        stop=(c == DC - 1),
                    )
                zst = nrm.tile([R, T], f32, tag="zst")
                nc.scalar.activation(
                    out=zst[:], in_=zp[:], func=AF.Identity, bias=bred_sb[:], scale=1.0
                )
                nc.sync.dma_start(out=ZOUT[:][:, t0 : t0 + T], in_=zst[:])
                nc.scalar.activation(
                    out=zTb[:, t0 : t0 + T],
                    in_=zp[:],
                    func=AF.Identity,
                    bias=bred_sb[:],
                    scale=1.0,
                )
                if with_mask:
                    nc.vector.tensor_mul(
                        zTbm[:, t0 : t0 + T],
                        zTb[:, t0 : t0 + T],
                        mask_sb[0:R, t0 : t0 + T],
                    )

            ga_sb = wts.tile([R, PLU], bf16, tag="ga")
            nc.sync.dma_start(out=ga_sb[:], in_=GA[:])
            gb_sb = wts.tile([R, PLU], bf16, tag="gb")
            nc.sync.dma_start(out=gb_sb[:], in_=GB[:])
            ones_sb = wts.tile([R, 128], bf16, tag="ones")
            nc.vector.memset(ones_sb[:], 1.0)
            w1_sb = wts.tile([PK, 12, HID], bf16, tag="w1")
            nc.sync.dma_start(
                out=w1_sb[:], in_=W1T[:].rearrange("(k p) h -> p k h", p=PK)
            )
            b1_sb = wts.tile([128, 4], f32, tag="b1")
            nc.sync.dma_start(
                out=b1_sb[:], in_=B1[:].rearrange("(m p) o -> p (m o)", p=128)
            )
            w2_sb = wts.tile([128, 4, D], bf16, tag="w2")
            nc.sync.dma_start(
                out=w2_sb[:], in_=W2T[:].rearrange("(k p) d -> p k d", p=128)
            )
            b2_sb = wts.tile([128, DC], f32, tag="b2")
            nc.sync.dma_start(
                out=b2_sb[:], in_=B2[:].rearrange("(m p) o -> p (m o)", p=128)
            )
            c1_sb = wts.tile([PK, NPK, HID], bf16, tag="c1")
            nc.sync.dma_start(
                out=c1_sb[:], in_=C1T[:].rearrange("(k p) h -> p k h", p=PK)
            )
            cb1_sb = wts.tile([128, 4], f32, tag="cb1")
            nc.sync.dma_start(
                out=cb1_sb[:], in_=CB1[:].rearrange("(m p) o -> p (m o)", p=128)
            )
            c2_sb = wts.tile([128, 4, D], bf16, tag="c2")
            nc.sync.dma_start(
                out=c2_sb[:], in_=C2T[:].rearrange("(k p) d -> p k d", p=128)
            )
            cb2_sb = wts.tile([128, DC], f32, tag="cb2")
            nc.sync.dma_start(
                out=cb2_sb[:], in_=CB2[:].rearrange("(m p) o -> p (m o)", p=128)
            )

            def emit_main(t):
                t0 = t * T
                # gathers: ZA[k,l]=z[ia_k,l], ZB[k,l]=z[ib_k,l] (masked z)
                zarrs = {"a": [], "b": []}
                for nm, gsb in (("a", ga_sb), ("b", gb_sb)):
                    for mk in range(NPK):
                        pm_ = psp.tile([PK, T], f32, tag="mm")
                        nc.tensor.matmul(
                            pm_[:],
                            gsb[:, mk * PK : (mk + 1) * PK],
                            zTbm[:, t0 : t0 + T],
                            start=True,
                            stop=True,
                        )
                        pt_ = pstp.tile([PK, 4], f32, tag="mmt")
                        nc.tensor.matmul(
                            pt_[:],
                            gsb[:, mk * PK : (mk + 1) * PK],
                            zTbm[:, t0 + T : t0 + T + 4],
                            start=True,
                            stop=True,
                        )
                        g = gath.tile([PK, T + 4], bf16, tag=f"g{nm}{mk}")
                        nc.scalar.activation(out=g[:, 0:T], in_=pm_[:], func=AF.Copy)
                        nc.scalar.activation(
                            out=g[:, T : T + 4], in_=pt_[:], func=AF.Copy
                        )
                        zarrs[nm].append(g)
                ZAt, ZBt = zarrs["a"], zarrs["b"]
                # norms via Lagrange identity (raw z)
                z2 = nrm.tile([R, T + 4], bf16, tag="z2")
                nc.vector.tensor_mul(
                    z2[:], zTb[:, t0 : t0 + T + 4], zTb[:, t0 : t0 + T + 4]
                )
                s2ps = psp.tile([128, T], f32, tag="mm")
                nc.tensor.matmul(s2ps[:], ones_sb[:], z2[:, 0:T], start=True, stop=True)
                s2sb = nrm.tile([128, T], f32, tag="s2sb")
                nc.scalar.activation(out=s2sb[:], in_=s2ps[:], func=AF.Copy)
                Fs = {}
                for d in OFFS:
                    dm = nrm.tile([R, T], bf16, tag="dm")
                    nc.vector.tensor_mul(
                        dm[:], zTb[:, t0 : t0 + T], zTb[:, t0 + d : t0 + d + T]
                    )
                    ssps = psp.tile([128, T], f32, tag="mm")
                    nc.tensor.matmul(
                        ssps[:], ones_sb[:], z2[:, d : d + T], start=True, stop=True
                    )
                    dps = psp.tile([128, T], f32, tag="mm")
                    nc.tensor.matmul(dps[:], ones_sb[:], dm[:], start=True, stop=True)
                    v = scr.tile([128, T], f32, tag="scr")
                    nc.vector.tensor_mul(v[:], s2sb[:], ssps[:])
                    e = scr.tile([128, T], f32, tag="scr")
                    nc.scalar.activation(out=e[:], in_=dps[:], func=AF.Square)
                    nc.vector.tensor_sub(v[:], v[:], e[:])
                    nc.vector.tensor_scalar_max(v[:], v[:], 1e-16)
                    r = scr.tile([128, T], f32, tag="scr")
                    nc.vector.reciprocal_approx_fast(r[:], v[:])
                    Ft = fpl.tile([128, T], bf16, tag=f"F{d}")
                    nc.scalar.activation(out=Ft[:], in_=r[:], func=AF.Sqrt)
                    Fs[d] = Ft
                # plucker products, normalize, write p tiles
                pms = {2: [], 4: []}
                for d in OFFS:
                    for mk in range(NPK):
                        t1 = plk.tile([PK, T], bf16, tag="t1")
                        nc.vector.tensor_mul(
                            t1[:], ZAt[mk][:, 0:T], ZBt[mk][:, d : d + T]
                        )
                        t2 = plk.tile([PK, T], bf16, tag="t2")
                        nc.vector.tensor_mul(
                            t2[:], ZBt[mk][:, 0:T], ZAt[mk][:, d : d + T]
                        )
                        nc.vector.tensor_sub(t1[:], t1[:], t2[:])
                        if d == 1:
                            dest = p1t[mk][:, t0 + 1 : t0 + T + 1]
                        else:
                            pm = pmt.tile([PK, T], bf16, tag=f"pm{d}_{mk}")
                            pms[d].append(pm)
                            dest = pm[:]
                        nc.vector.tensor_mul(dest, t1[:], Fs[d][0:PK, :])
                # MLP1+MLP2 (backbone)
                rhs = [p1t[mk][:, t0 + 1 : t0 + T + 1] for mk in range(NPK)]
                rhs += [pm[:] for pm in pms[2]]
                rhs += [pm[:] for pm in pms[4]]
                g1s = []
                for m in range(4):
                    y = psp.tile([128, T], f32, tag="mm")
                    for kc in range(12):
                        nc.tensor.matmul(
                            y[:],
                            w1_sb[:, kc, m * 128 : (m + 1) * 128],
                            rhs[kc],
                            start=(kc == 0),
                            stop=(kc == 11),
                        )
                    g1 = g1p.tile([128, T], bf16, tag="g1")
                    nc.scalar.activation(
                        out=g1[:],
                        in_=y[:],
                        func=AF.Gelu,
                        bias=b1_sb[:, m : m + 1],
                        scale=1.0,
                    )
                    g1s.append(g1)
                gstage = stg.tile([128, DC, T], bf16, tag="gst")
                for m in range(DC):
                    y2 = psp.tile([128, T], f32, tag="mm")
                    for k in range(4):
                        nc.tensor.matmul(
                            y2[:],
                            w2_sb[:, k, m * 128 : (m + 1) * 128],
                            g1s[k][:],
                            start=(k == 0),
                            stop=(k == 3),
                        )
                    if m % 2 == 0:
                        nc.scalar.activation(
                            out=gstage[:, m, :],
                            in_=y2[:],
                            func=AF.Identity,
                            bias=b2_sb[:, m : m + 1],
                            scale=1.0,
                        )
                    else:
                        nc.vector.tensor_scalar_add(
                            gstage[:, m, :], y2[:], b2_sb[:, m : m + 1]
                        )
                nc.sync.dma_start(
                    out=GBOUT[:].rearrange("(c p) l -> p c l", p=128)[
                        :, :, t0 : t0 + T
                    ],
                    in_=gstage[:],
                )

            def emit_curv(tp):
                tp0 = tp * T
                kst = stg.tile([PK, NPK, T], bf16, tag="kst")
                for mk in range(NPK):
                    ka = kst[:, mk, :]
                    nc.vector.tensor_add(
                        ka,
                        p1t[mk][:, tp0 + 2 : tp0 + T + 2],
                        p1t[mk][:, tp0 : tp0 + T],
                    )
                    nc.vector.scalar_tensor_tensor(
                        out=ka,
                        in0=p1t[mk][:, tp0 + 1 : tp0 + T + 1],
                        scalar=-2.0,
                        in1=ka,
                        op0=ALU.mult,
                        op1=ALU.add,
                    )
                    if with_mask:
                        nc.vector.tensor_mul(ka, ka, mask_sb[0:PK, tp0 : tp0 + T])
                nc.sync.dma_start(
                    out=KOUT[:].rearrange("(c p) l -> p c l", p=PK)[
                        :, :, tp0 : tp0 + T
                    ],
                    in_=kst[:],
                )
                g1s = []
                for m in range(4):
                    y = psp.tile([128, T], f32, tag="mm")
                    for kc in range(NPK):
                        nc.tensor.matmul(
                            y[:],
                            c1_sb[:, kc, m * 128 : (m + 1) * 128],
                            kst[:, kc, :],
                            start=(kc == 0),
                            stop=(kc == NPK - 1),
                        )
                    g1 = g1p.tile([128, T], bf16, tag="g1")
                    nc.scalar.activation(
                        out=g1[:],
                        in_=y[:],
                        func=AF.Gelu,
                        bias=cb1_sb[:, m : m + 1],
                        scale=1.0,
                    )
                    g1s.append(g1)
                cstage = stg.tile([128, DC, T], bf16, tag="cst")
                for m in range(DC):
                    y2 = psp.tile([128, T], f32, tag="mm")
                    for k in range(4):
                        nc.tensor.matmul(
                            y2[:],
                            c2_sb[:, k, m * 128 : (m + 1) * 128],
                            g1s[k][:],
                            start=(k == 0),
                            stop=(k == 3),
                        )
                    if m % 2 == 0:
                        nc.scalar.activation(
                            out=cstage[:, m, :],
                            in_=y2[:],
                            func=AF.Identity,
                            bias=cb2_sb[:, m : m + 1],
                            scale=1.0,
                        )
                    else:
                        nc.vector.tensor_scalar_add(
                            cstage[:, m, :], y2[:], cb2_sb[:, m : m + 1]
                        )
                nc.sync.dma_start(
                    out=GCOUT[:].rearrange("(c p) l -> p c l", p=128)[
                        :, :, tp0 : tp0 + T
                    ],
                    in_=cstage[:],
                )

            for t in range(NT):
                emit_main(t)
                if t >= 1:
                    emit_curv(t - 1)
            emit_curv(NT - 1)

            for mk in range(NPK):
                nc.sync.dma_start(
                    out=P1OUT[:][mk * PK : (mk + 1) * PK, :],
                    in_=p1t[mk][:, 1 : L + 1],
                )

    nc.compile()
    return nc


def _prep_maps(h, seq_mask, w_red, b_red, bb_w1, bb_b1, bb_w2, bb_b2, cv_w1, cv_b1,
               cv_w2, cv_b2, with_mask):
    f = lambda x: np.ascontiguousarray(np.asarray(x, np.float32))
    bb = lambda x: np.ascontiguousarray(np.asarray(x, np.float32)).astype(BF)
    ia, ib = np.triu_indices(R, k=1)
    ga = np.zeros((R, PLU), np.float32)
    gb = np.zeros((R, PLU), np.float32)
    ga[ia, np.arange(PLU)] = 1.0
    gb[ib, np.arange(PLU)] = 1.0
    shared = {
        "wredT": bb(f(w_red).T),
        "bred": f(b_red).reshape(R, 1),
        "ga": ga.astype(BF),
        "gb": gb.astype(BF),
        "w1T": bb(f(bb_w1).T),
        "b1": f(bb_b1).reshape(HID, 1),
        "w2T": bb(f(bb_w2).T),
        "b2": f(bb_b2).reshape(D, 1),
        "c1T": bb(f(cv_w1).T),
        "cb1": f(cv_b1).reshape(HID, 1),
        "c2T": bb(f(cv_w2).T),
        "cb2": f(cv_b2).reshape(D, 1),
    }
    h = np.asarray(h, np.float32)
    maps = []
    for i in range(B):
        m = dict(shared)
        m["hT"] = np.ascontiguousarray(h[i].T).astype(BF)
        if with_mask:
            mf = np.zeros((LP,), np.float32)
            mf[:L] = np.asarray(seq_mask[i], np.float32)
            m["mask128"] = np.broadcast_to(mf[None, :], (128, LP)).astype(BF)
        maps.append(m)
    return maps


def kernel(h, seq_mask, w_red, b_red, bb_w1, bb_b1, bb_w2, bb_b2, cv_w1, cv_b1,
           cv_w2, cv_b2):
    global LAST_RESULT
    mask = np.asarray(seq_mask).astype(bool)
    with_mask = not bool(mask.all())
    nc = _CACHE.get(with_mask)
    if nc is None:
        nc = _build(with_mask)
        _CACHE[with_mask] = nc
    maps = _prep_maps(h, seq_mask, w_red, b_red, bb_w1, bb_b1, bb_w2, bb_b2,
                      cv_w1, cv_b1, cv_w2, cv_b2, with_mask)
    kw = {}
    if TRACE:
        kw = dict(trace=True, tmpdir=TRACE_DIR)
    res = run_bass_kernel_spmd(nc, maps, list(range(B)), **kw)
    LAST_RESULT = res
    rs = res.results
    z = np.stack([np.asarray(rs[i]["zT_out"], np.float32).T for i in range(B)])
    g_bb = np.stack(
        [np.asarray(rs[i]["gbT_out"]).T.astype(np.float32) for i in range(B)]
    )
    g_curv = np.stack(
        [np.asarray(rs[i]["gcT_out"]).T.astype(np.float32) for i in range(B)]
    )
    p_bb1 = np.stack(
        [np.asarray(rs[i]["p1T_out"]).T.astype(np.float32) for i in range(B)]
    )
    kappa = np.stack(
        [np.asarray(rs[i]["kT_out"]).T.astype(np.float32) for i in range(B)]
    )
    return z, g_bb, g_curv, p_bb1, kappa
